# revision 51
# baseline (speedup 1.0000x reference)
"""Causal multi-head attention (B=4, S=2048, D=1024, H=16, Hd=64) on 8 TRN2
NeuronCores.

Sharding: tensor-parallel over heads. Core c owns heads [2c, 2c+1]:
  - Wq/Wk/Wv column-sharded (rows of the [out,in] weight): each core projects
    x -> qT/kT/vT [128, S] (2 heads x 64, head-dim-major).
  - Attention per (b, h) computed entirely on-core, scoresT layout
    [keys, queries] so softmax normalization folds into matmuls (ones row in
    the extended V gives the denominator for free).
  - Wo row-sharded: each core emits a partial [B,S,D] output; host sums the
    8 partials.

Perf structure:
  - Causal trimming: for diagonal key-strips only the valid query range
    [r, 512) is computed in scores / streamed into AV; the mask is a single
    [128,128] lower-triangular block multiply.
  - Scores PSUM is per-STRIP [128, 2(heads), 512]: one release event (the
    strip's exp) frees both heads' banks together, so the next pair's
    row-tiled head matmuls become ready simultaneously and run concurrently
    on disjoint PE row groups.
  - Projection and output-projection matmuls are emitted as small work items
    woven between attention strip-pairs, so the PE never idles while ACT
    works through the exp stream; a few items are held in reserve to bridge
    batch boundaries (keeps the PE HAM clock at 2.4GHz).
  - Last batch: outproj folds are delayed one query-chunk and spread over
    the tail pairs; the final normalization multiplies are split into
    128-col pieces so outproj starts per-block.

Numerics: matmul operands in bf16 (fp32 PSUM accumulation), softmax without
max-subtraction (scores are bounded ~|10| for this data distribution), causal
mask applied post-exp as a {0,1} triangular multiply.
"""

import os
from collections import deque
import numpy as np
import ml_dtypes
from contextlib import ExitStack

import concourse.bass as bass
import concourse.tile as tile
from concourse import bacc, mybir
from concourse.bass_utils import run_bass_kernel_spmd
from concourse.masks import make_identity

F32 = mybir.dt.float32
BF16 = mybir.dt.bfloat16
NPBF16 = ml_dtypes.bfloat16

B, S, D = 4, 2048, 1024
H, HD = 16, 64
NCORES = 8
HPC = H // NCORES          # heads per core
DH = HPC * HD              # local head dim (128)
TC = 512                   # token chunk for projections / query chunk
KS = 128                   # key strip

last_exec_time_ns = None   # set by kernel() when BASS_TRACE=1


def emit(tc_ctx: tile.TileContext, ctx: ExitStack, aps: dict, b_count: int, seq: int):
    """Emit the per-core program. aps: xt [b,D,seq] bf16, wq/wk/wv [D,DH] bf16,
    wo [DH,D] bf16, mask [128, 128] bf16 (lower-tri incl), out [b,seq,D] bf16."""
    nc = tc_ctx.nc
    tc = tc_ctx
    KC = D // 128            # contraction chunks for projections
    NTC = seq // TC          # token chunks
    NQC = seq // TC          # query chunks
    NKS = seq // KS          # key strips

    xt, wq, wk, wv, wo, mask, out = (
        aps["xt"], aps["wq"], aps["wk"], aps["wv"], aps["wo"], aps["mask"], aps["out"]
    )

    wpool = ctx.enter_context(tc.tile_pool(name="wpool", bufs=1))
    xpool = ctx.enter_context(tc.tile_pool(name="xpool", bufs=4))
    qkpool = ctx.enter_context(tc.tile_pool(name="qkpool", bufs=4))
    vpool = ctx.enter_context(tc.tile_pool(name="vpool", bufs=2))
    ppool = ctx.enter_context(tc.tile_pool(name="ppool", bufs=6))
    avpool = ctx.enter_context(tc.tile_pool(name="avpool", bufs=4))
    smalls = ctx.enter_context(tc.tile_pool(name="smalls", bufs=4))

    ps_scr = ctx.enter_context(tc.tile_pool(name="ps_scr", bufs=2, space="PSUM"))
    ps_p = ctx.enter_context(tc.tile_pool(name="ps_p", bufs=2, space="PSUM"))
    ps_av = ctx.enter_context(tc.tile_pool(name="ps_av", bufs=2, space="PSUM"))

    # --- first input chunk ASAP: the xt (b0, tcc0) DMAs gate the first matmul,
    # so they go out before the ~26 weight/mask dma_starts (each dma_start
    # costs ~0.6us of serial sequencing).
    xt_tiles = {}

    def emit_xt_dma(b, tcc, split=False):
        xt_src = xt[b].rearrange("(kc p) t -> p kc t", p=128)
        ts = []
        for kc in range(KC):
            t = xpool.tile([128, TC], BF16, tag="xt",
                           name=f"xt_{b}_{tcc}_{kc}", bufs=20)
            eng = nc.scalar if (split and kc % 2) else nc.sync
            eng.dma_start(out=t, in_=xt_src[:, kc, tcc * TC:(tcc + 1) * TC])
            ts.append(t)
        xt_tiles[(b, tcc)] = ts

    emit_xt_dma(0, 0, split=True)

    # --- constants / weights: per-(name, kc-half) tiles [128,4,DH]; six
    # 128KB DMAs instead of 24 small ones keeps sequencing cost low while
    # letting the first half of each name land early ---
    w_sb = {}
    for half in range(2):
        for name, ap in (("wq", wq), ("wk", wk), ("wv", wv)):
            t = wpool.tile([128, 4, DH], BF16, tag=f"w_{name}_{half}",
                           name=f"w_{name}_{half}")
            src_ap = ap.rearrange("(kc p) m -> p kc m", p=128)
            nc.scalar.dma_start(out=t, in_=src_ap[:, half * 4:half * 4 + 4, :])
            w_sb[(name, half)] = t
    mask_sb = wpool.tile([128, 128], BF16)
    nc.scalar.dma_start(out=mask_sb, in_=mask)
    wo_sb = wpool.tile([128, D], BF16)
    nc.scalar.dma_start(out=wo_sb, in_=wo)
    for _tcc in range(1, NTC):
        emit_xt_dma(0, _tcc, split=True)

    ident_f = wpool.tile([128, 64], F32)
    make_identity(nc, ident_f[0:64, :])
    make_identity(nc, ident_f[64:128, :])
    ident = wpool.tile([128, 64], BF16)
    nc.vector.tensor_copy(ident, ident_f)

    ones_f = wpool.tile([128, 64], F32)
    nc.vector.memset(ones_f, 1.0)
    ones_r = wpool.tile([128, 64], BF16)
    nc.vector.tensor_copy(ones_r, ones_f)


    qTs, kTs, vexts, avTs = {}, {}, {}, {}

    def alloc_batch(b):
        qTs[b] = qkpool.tile([128, seq], BF16, tag="qT", name=f"qT{b}")
        kTs[b] = qkpool.tile([128, seq], BF16, tag="kT", name=f"kT{b}")
        vexts[(b, "vT")] = vpool.tile([128, seq], BF16, tag="vT", name=f"vT{b}")
        vexts[b] = vpool.tile([128, HPC, NKS, 65], BF16, tag="vext",
                              name=f"vext{b}", bufs=4)
        # static ones column (softmax denominator row) written once at alloc,
        # so AV matmuls never wait on the last projection chunk
        for h in range(HPC):
            nc.vector.tensor_copy(vexts[b][:, h, :, 64:65],
                                  ones_r[:, 0:1].to_broadcast([128, NKS, 1]))

    # ---------------- projection work items ----------------
    def proj_items(b):
        """Work-item closures for batch b's projections: DMA, 4-MM bursts,
        casts, v-transposes. Each item is a small dense PE burst."""
        items = []
        ps_live = {}

        def mm_half(name, tcc, half, b=b):
            def run():
                xs = xt_tiles[(b, tcc)]
                if half == 0:
                    ps = ps_scr.tile([128, TC], F32, tag="scr", name=f"ps_{name}")
                    ps_live[(name, tcc)] = ps
                else:
                    ps = ps_live.pop((name, tcc))
                for kc in range(half * 4, half * 4 + 4):
                    nc.tensor.matmul(ps, w_sb[(name, half)][:, kc % 4, :], xs[kc],
                                     start=(kc == 0), stop=(kc == KC - 1))
                if half == 1:
                    dstmap = {"wq": qTs[b], "wk": kTs[b], "wv": vexts[(b, "vT")]}
                    # high priority: this cast releases the PSUM slot the next
                    # projection burst needs — don't let it queue behind other
                    # DVE work
                    with tc.high_priority(offset=30):
                        nc.vector.tensor_copy(
                            dstmap[name][:, tcc * TC:(tcc + 1) * TC], ps)
            return run

        def post_item(tcc, b=b):
            def run():
                vT = vexts[(b, "vT")]
                vext = vexts[b]
                for h in range(HPC):
                    tr4 = ps_scr.tile([128, 4, 64], BF16, tag="scr", name="tr4")
                    for i in range(4):
                        ks = tcc * 4 + i
                        nc.tensor.transpose(
                            tr4[:, i, :],
                            vT[h * 64:(h + 1) * 64, ks * 128:(ks + 1) * 128],
                            ident[h * 64:(h + 1) * 64, :])
                    nc.vector.tensor_copy(vext[:, h, tcc * 4:(tcc + 1) * 4, 0:64], tr4)
            return run

        for tcc in range(NTC):
            if b != 0:  # batch 0's xt DMAs all go out at program start
                items.append(lambda b=b, tcc=tcc: emit_xt_dma(b, tcc))
            for name in ("wq", "wk", "wv"):
                items.append(mm_half(name, tcc, 0))
                items.append(mm_half(name, tcc, 1))
            items.append(post_item(tcc))
        return items

    # ---------------- output projection work items ----------------
    def emit_outproj_t16(b, t16):
        avT = avTs[b]
        o_sb = smalls.tile([128, 2, TC], BF16, tag="o")
        for n2 in range(D // TC):
            po = ps_scr.tile([128, TC], F32, tag="scr", name="po")
            nc.tensor.matmul(po, avT[:, t16 * 128:(t16 + 1) * 128],
                             wo_sb[:, n2 * TC:(n2 + 1) * TC],
                             start=True, stop=True)
            if (t16 + n2) % 2 == 0:
                nc.vector.tensor_copy(o_sb[:, n2, :], po)
            else:
                nc.scalar.copy(o_sb[:, n2, :], po)
        nc.sync.dma_start(
            out=out[b, t16 * 128:(t16 + 1) * 128, :],
            in_=o_sb.rearrange("p a b -> p (a b)"))

    def outproj_items(b):
        return [lambda b=b, t16=t16: emit_outproj_t16(b, t16)
                for t16 in range(seq // 128)]

    # ---------------- attention ----------------
    av_state = {}

    def emit_attn_pair(b, qc, g):
        """One strip-pair (strips 2g, 2g+1) of attention for batch b, both
        heads. j0 = higher strip (more causally trimmed) so the exp range is
        one contiguous tail slice."""
        qT, kT, vext = qTs[b], kTs[b], vexts[b]
        nstrips = 4 * qc + 4
        # j0 = strip 2g+1 (larger r), j1 = strip 2g
        order = []
        for j, st in ((0, 2 * g + 1), (1, 2 * g)):
            r = st * KS - qc * TC
            order.append((j, st, max(r, 0), r >= 0))
        # one PSUM tile per STRIP covering both heads: a single release event
        # (the strip's exp) frees both heads' banks together, so the next
        # pair's row-tiled head matmuls become ready simultaneously and join.
        pps = {}
        p_sbs = {}
        for j, st, r, diag in order:
            pps[j] = ps_p.tile([128, 2, TC], F32, tag="pp", name=f"pp{j}")
            for h in range(HPC):
                nc.tensor.matmul(
                    pps[j][:, h, r:],
                    kT[h * 64:(h + 1) * 64, st * KS:(st + 1) * KS],
                    qT[h * 64:(h + 1) * 64, qc * TC + r:(qc + 1) * TC],
                    start=True, stop=True)
            p_sb = ppool.tile([128, 2, TC], BF16, tag="p", name=f"p{j}")
            p_sbs[j] = p_sb
            nc.scalar.activation(
                p_sb.rearrange("p a b -> p (a b)"),
                pps[j].rearrange("p a b -> p (a b)"),
                mybir.ActivationFunctionType.Exp)
            if diag:  # triangular block multiply on the diagonal 128 cols
                for h in range(HPC):
                    nc.vector.tensor_mul(p_sb[:, h, r:r + KS],
                                         p_sb[:, h, r:r + KS], mask_sb)
        # For the first pair of a qc, issue the full-width strip (2g) first so
        # start=True covers every column of the accumulator.
        av_order = order[::-1] if g == 0 else order
        for j, st, r, diag in av_order:
            first = av_state[(b, qc)] == 0
            av_state[(b, qc)] += 1
            last_issue = av_state[(b, qc)] == nstrips
            for h in range(HPC):
                nc.tensor.matmul(pav_cur[h][:, r:], vext[:, h, st, :],
                                 p_sbs[j][:, h, r:],
                                 start=first, stop=last_issue)

    def emit_qc_norm(b, qc, split_mul=False):
        """Per-(head) softmax normalization for this query chunk. With
        split_mul, the final multiply is emitted in 128-col pieces so each
        downstream outproj block starts as soon as its columns are ready."""
        avT = avTs[b]
        for h in range(HPC):
            ave = smalls.tile([65, TC], F32, tag="ave")
            nc.vector.tensor_copy(ave, pav_cur[h])
            z0 = smalls.tile([1, TC], F32, tag="z0")
            nc.sync.dma_start(out=z0, in_=ave[64:65, :])
            zbb = smalls.tile([64, TC], F32, tag="zbb")
            nc.gpsimd.partition_broadcast(zbb, z0)
            rz = smalls.tile([64, TC], F32, tag="rz")
            nc.vector.reciprocal_approx_fast(rz, zbb)
            pieces = range(0, TC, 128) if split_mul else (0,)
            width = 128 if split_mul else TC
            with nc.allow_low_precision(reason="attn weights tolerate bf16"):
                for c0 in pieces:
                    nc.vector.tensor_mul(
                        avT[h * 64:(h + 1) * 64,
                            qc * TC + c0:qc * TC + c0 + width],
                        ave[0:64, c0:c0 + width], rz[:, c0:c0 + width])

    # ---------------- main schedule ----------------
    # batch 0 projections run solo up front
    alloc_batch(0)
    for it in proj_items(0):
        it()

    items = deque()
    deferred = []
    for b in range(b_count):
        avTs[b] = avpool.tile([128, seq], BF16, tag="avT", name=f"avT{b}")
        items.extend(deferred)
        deferred = []
        if b + 1 < b_count:
            alloc_batch(b + 1)
            items.extend(proj_items(b + 1))
        last = b == b_count - 1
        n_items = len(items)
        total_pairs = sum(2 * qc + 2 for qc in range(NQC))
        pair_idx = 0
        popped = 0
        for qc in range(NQC):
            av_state[(b, qc)] = 0
            global pav_cur
            pav_cur = {h: ps_av.tile([65, TC], F32, tag="av", name=f"pav{h}")
                       for h in range(HPC)}
            for g in range(2 * qc + 2):
                emit_attn_pair(b, qc, g)
                pair_idx += 1
                npairs = 2 * qc + 2
                if last and qc > 0 and g >= npairs - 4:
                    # previous qc's outproj folds into this qc's tail pairs:
                    # it keeps the PE warm through the norm-chain bubble
                    t4 = g - (npairs - 4)
                    emit_outproj_t16(b, (qc - 1) * (TC // 128) + t4)
                target = n_items * pair_idx // (total_pairs + 8)
                while popped < target and items:
                    items.popleft()()
                    popped += 1
            emit_qc_norm(b, qc, split_mul=last)
        while items:  # safety drain
            items.popleft()()
        if last:
            for t4 in range(TC // 128):
                emit_outproj_t16(b, (NQC - 1) * (TC // 128) + t4)
        if not last:
            # split the outproj weave across the next two batches: early
            # batches' attention phases are PE-oversubscribed while the last
            # batch's is ACT-bound with spare PE cycles
            oi = outproj_items(b)
            cut = len(oi) if b + 2 >= b_count else 10
            items.extend(oi[:cut])
            deferred.extend(oi[cut:])


def host_inputs(x, Wq, Wk, Wv, Wo, core, xt_bf=None):
    """Build the per-core input map."""
    hs = slice(core * DH, (core + 1) * DH)
    if xt_bf is None:
        xt_bf = np.ascontiguousarray(np.transpose(x, (0, 2, 1))).astype(NPBF16)
    wq = np.ascontiguousarray((Wq[hs, :] * np.float32(1.0 / np.sqrt(HD))).T).astype(NPBF16)
    wk = np.ascontiguousarray(Wk[hs, :].T).astype(NPBF16)
    wv = np.ascontiguousarray(Wv[hs, :].T).astype(NPBF16)
    wo = np.ascontiguousarray(Wo[:, hs].T).astype(NPBF16)
    # lower-triangular inclusive [128,128]: valid (q_local >= k_local)
    mask = (np.arange(128)[None, :] >= np.arange(128)[:, None]).astype(NPBF16)
    return {"xt": xt_bf, "wq": wq, "wk": wk, "wv": wv, "wo": wo, "mask": mask}


def build_program(b_count=B, seq=S):
    nc = bacc.Bacc("TRN2", target_bir_lowering=False, debug=False,
                   num_devices=NCORES)
    aps = {
        "xt": nc.dram_tensor("xt", [b_count, D, seq], BF16, kind="ExternalInput").ap(),
        "wq": nc.dram_tensor("wq", [D, DH], BF16, kind="ExternalInput").ap(),
        "wk": nc.dram_tensor("wk", [D, DH], BF16, kind="ExternalInput").ap(),
        "wv": nc.dram_tensor("wv", [D, DH], BF16, kind="ExternalInput").ap(),
        "wo": nc.dram_tensor("wo", [DH, D], BF16, kind="ExternalInput").ap(),
        "mask": nc.dram_tensor("mask", [128, 128], BF16, kind="ExternalInput").ap(),
        "out": nc.dram_tensor("out", [b_count, seq, D], BF16, kind="ExternalOutput").ap(),
    }
    with tile.TileContext(nc) as tcx:
        with ExitStack() as ctx:
            emit(tcx, ctx, aps, b_count, seq)
    nc.finalize()
    return nc


def _ensure_ntff_hook():
    """Register the ctypes NTFF profile hook when the image lacks
    antenv.axon_hooks (needed only for trace=True)."""
    import sys, types
    try:
        import antenv.axon_hooks  # noqa: F401
        return
    except ImportError:
        pass
    try:
        import antenv
        from trn_agent_boot.trn_boot import _ntff_profile_via_ctypes
        hook = _ntff_profile_via_ctypes("/opt/axon/libaxon_pjrt.so")
        mod = types.ModuleType("antenv.axon_hooks")
        mod.get_axon_ntff_profile_hook = lambda: hook
        mod.set_axon_ntff_profile_hook = lambda h: None
        sys.modules["antenv.axon_hooks"] = mod
        antenv.axon_hooks = mod
    except Exception:
        pass


def kernel(x, Wq, Wk, Wv, Wo):
    global last_exec_time_ns
    x = np.asarray(x, dtype=np.float32)
    Wq = np.asarray(Wq, dtype=np.float32)
    Wk = np.asarray(Wk, dtype=np.float32)
    Wv = np.asarray(Wv, dtype=np.float32)
    Wo = np.asarray(Wo, dtype=np.float32)

    nc = build_program(B, S)
    xt_bf = np.ascontiguousarray(np.transpose(x, (0, 2, 1))).astype(NPBF16)
    in_maps = [host_inputs(x, Wq, Wk, Wv, Wo, c, xt_bf=xt_bf) for c in range(NCORES)]
    trace = bool(os.environ.get("BASS_TRACE"))
    if trace:
        _ensure_ntff_hook()
    res = run_bass_kernel_spmd(nc, in_maps, list(range(NCORES)), trace=trace)
    last_exec_time_ns = res.exec_time_ns
    parts = [res.results[c]["out"] for c in range(NCORES)]
    acc = parts[0].astype(np.float32)
    for p in parts[1:]:
        acc = acc + p
    return acc


# revision 52
# speedup vs baseline: 1.1560x; 1.1560x over previous
"""Causal multi-head attention (B=4, S=2048, D=1024, H=16, Hd=64) on 8 TRN2
NeuronCores.

Sharding: tensor-parallel over heads. Core c owns heads [2c, 2c+1]:
  - Wq/Wk/Wv column-sharded (rows of the [out,in] weight): each core projects
    x -> qT/kT/vT [128, S] (2 heads x 64, head-dim-major).
  - Attention per (b, h) computed entirely on-core, scoresT layout
    [keys, queries] so softmax normalization folds into matmuls (ones row in
    the extended V gives the denominator for free).
  - Wo row-sharded: each core emits a partial [B,S,D] output; host sums the
    8 partials.

Perf structure:
  - Causal trimming: for diagonal key-strips only the valid query range
    [r, 512) is computed in scores / streamed into AV; the mask is a single
    [128,128] lower-triangular block multiply.
  - Scores PSUM is per-STRIP [128, 2(heads), 512]: one release event (the
    strip's exp) frees both heads' banks together, so the next pair's
    row-tiled head matmuls become ready simultaneously and run concurrently
    on disjoint PE row groups.
  - Projection and output-projection matmuls are emitted as small work items
    woven between attention strip-pairs, so the PE never idles while ACT
    works through the exp stream; a few items are held in reserve to bridge
    batch boundaries (keeps the PE HAM clock at 2.4GHz).
  - Last batch: outproj folds are delayed one query-chunk and spread over
    the tail pairs; the final normalization multiplies are split into
    128-col pieces so outproj starts per-block.

Numerics: matmul operands in bf16 (fp32 PSUM accumulation), softmax without
max-subtraction (scores are bounded ~|10| for this data distribution), causal
mask applied post-exp as a {0,1} triangular multiply.
"""

import os
from collections import deque
import numpy as np
import ml_dtypes
from contextlib import ExitStack

import concourse.bass as bass
import concourse.tile as tile
from concourse import bacc, mybir
from concourse.bass_utils import run_bass_kernel_spmd
from concourse.masks import make_identity

F32 = mybir.dt.float32
BF16 = mybir.dt.bfloat16
NPBF16 = ml_dtypes.bfloat16

B, S, D = 4, 2048, 1024
H, HD = 16, 64
NCORES = 8
HPC = H // NCORES          # heads per core
DH = HPC * HD              # local head dim (128)
TC = 512                   # token chunk for projections / query chunk
KS = 128                   # key strip

last_exec_time_ns = None   # set by kernel() when BASS_TRACE=1


def emit(tc_ctx: tile.TileContext, ctx: ExitStack, aps: dict, b_count: int, seq: int):
    """Emit the per-core program. aps: xt [b,D,seq] bf16, wq/wk/wv [D,DH] bf16,
    wo [DH,D] bf16, mask [128, 128] bf16 (lower-tri incl), out [b,seq,D] bf16."""
    nc = tc_ctx.nc
    tc = tc_ctx
    KC = D // 128            # contraction chunks for projections
    NTC = seq // TC          # token chunks
    NQC = seq // TC          # query chunks
    NKS = seq // KS          # key strips

    xt, wq, wk, wv, wo, mask, out = (
        aps["xt"], aps["wq"], aps["wk"], aps["wv"], aps["wo"], aps["mask"], aps["out"]
    )

    wpool = ctx.enter_context(tc.tile_pool(name="wpool", bufs=1))
    xpool = ctx.enter_context(tc.tile_pool(name="xpool", bufs=4))
    qkpool = ctx.enter_context(tc.tile_pool(name="qkpool", bufs=4))
    vpool = ctx.enter_context(tc.tile_pool(name="vpool", bufs=2))
    ppool = ctx.enter_context(tc.tile_pool(name="ppool", bufs=4))
    avpool = ctx.enter_context(tc.tile_pool(name="avpool", bufs=4))
    smalls = ctx.enter_context(tc.tile_pool(name="smalls", bufs=4))

    ps_scr = ctx.enter_context(tc.tile_pool(name="ps_scr", bufs=2, space="PSUM"))
    ps_p = ctx.enter_context(tc.tile_pool(name="ps_p", bufs=2, space="PSUM"))
    ps_av = ctx.enter_context(tc.tile_pool(name="ps_av", bufs=2, space="PSUM"))

    # --- first input chunk ASAP: the xt (b0, tcc0) DMAs gate the first matmul,
    # so they go out before the ~26 weight/mask dma_starts (each dma_start
    # costs ~0.6us of serial sequencing).
    xt_tiles = {}

    def emit_xt_dma(b, tcc, split=False):
        xt_src = xt[b].rearrange("(kc p) t -> p kc t", p=128)
        ts = []
        for kc in range(KC):
            t = xpool.tile([128, TC], BF16, tag="xt",
                           name=f"xt_{b}_{tcc}_{kc}", bufs=20)
            eng = nc.scalar if (split and kc % 2) else nc.sync
            eng.dma_start(out=t, in_=xt_src[:, kc, tcc * TC:(tcc + 1) * TC])
            ts.append(t)
        xt_tiles[(b, tcc)] = ts

    emit_xt_dma(0, 0, split=True)

    # --- constants / weights: per-(name, kc-half) tiles [128,4,DH]; six
    # 128KB DMAs instead of 24 small ones keeps sequencing cost low while
    # letting the first half of each name land early ---
    w_sb = {}
    for half in range(2):
        for name, ap in (("wq", wq), ("wk", wk), ("wv", wv)):
            t = wpool.tile([128, 4, DH], BF16, tag=f"w_{name}_{half}",
                           name=f"w_{name}_{half}")
            src_ap = ap.rearrange("(kc p) m -> p kc m", p=128)
            nc.scalar.dma_start(out=t, in_=src_ap[:, half * 4:half * 4 + 4, :])
            w_sb[(name, half)] = t
    mask_sb = wpool.tile([128, 128], BF16)
    nc.scalar.dma_start(out=mask_sb, in_=mask)
    wo_sb = wpool.tile([128, D], BF16)
    nc.scalar.dma_start(out=wo_sb, in_=wo)
    for _tcc in range(1, NTC):
        emit_xt_dma(0, _tcc, split=True)

    ident_f = wpool.tile([128, 64], F32)
    make_identity(nc, ident_f[0:64, :])
    make_identity(nc, ident_f[64:128, :])
    ident = wpool.tile([128, 64], BF16)
    nc.vector.tensor_copy(ident, ident_f)

    ones_f = wpool.tile([128, 64], F32)
    nc.vector.memset(ones_f, 1.0)
    ones_r = wpool.tile([128, 64], BF16)
    nc.vector.tensor_copy(ones_r, ones_f)


    qTs, kTs, vexts, avTs = {}, {}, {}, {}

    def alloc_batch(b):
        qTs[b] = qkpool.tile([128, seq], BF16, tag="qT", name=f"qT{b}")
        kTs[b] = qkpool.tile([128, seq], BF16, tag="kT", name=f"kT{b}")
        vexts[(b, "vT")] = vpool.tile([128, seq], BF16, tag="vT", name=f"vT{b}")
        vexts[b] = vpool.tile([128, HPC, NKS, 65], BF16, tag="vext",
                              name=f"vext{b}", bufs=4)
        # static ones column (softmax denominator row) written once at alloc,
        # so AV matmuls never wait on the last projection chunk
        for h in range(HPC):
            nc.vector.tensor_copy(vexts[b][:, h, :, 64:65],
                                  ones_r[:, 0:1].to_broadcast([128, NKS, 1]))

    # ---------------- projection work items ----------------
    def proj_items(b):
        """Work-item closures for batch b's projections: DMA, 4-MM bursts,
        casts, v-transposes. Each item is a small dense PE burst."""
        items = []
        ps_live = {}

        def mm_half(name, tcc, half, b=b):
            def run():
                xs = xt_tiles[(b, tcc)]
                if half == 0:
                    ps = ps_scr.tile([128, TC], F32, tag="scr", name=f"ps_{name}")
                    ps_live[(name, tcc)] = ps
                else:
                    ps = ps_live.pop((name, tcc))
                for kc in range(half * 4, half * 4 + 4):
                    nc.tensor.matmul(ps, w_sb[(name, half)][:, kc % 4, :], xs[kc],
                                     start=(kc == 0), stop=(kc == KC - 1))
                if half == 1:
                    dstmap = {"wq": qTs[b], "wk": kTs[b], "wv": vexts[(b, "vT")]}
                    # high priority: this cast releases the PSUM slot the next
                    # projection burst needs — don't let it queue behind other
                    # DVE work
                    with tc.high_priority(offset=30):
                        nc.vector.tensor_copy(
                            dstmap[name][:, tcc * TC:(tcc + 1) * TC], ps)
            return run

        def post_item(tcc, b=b):
            def run():
                vT = vexts[(b, "vT")]
                vext = vexts[b]
                for h in range(HPC):
                    tr4 = ps_scr.tile([128, 4, 64], BF16, tag="scr", name="tr4")
                    for i in range(4):
                        ks = tcc * 4 + i
                        nc.tensor.transpose(
                            tr4[:, i, :],
                            vT[h * 64:(h + 1) * 64, ks * 128:(ks + 1) * 128],
                            ident[h * 64:(h + 1) * 64, :])
                    nc.vector.tensor_copy(vext[:, h, tcc * 4:(tcc + 1) * 4, 0:64], tr4)
            return run

        for tcc in range(NTC):
            if b != 0:  # batch 0's xt DMAs all go out at program start
                items.append(lambda b=b, tcc=tcc: emit_xt_dma(b, tcc))
            for name in ("wq", "wk", "wv"):
                items.append(mm_half(name, tcc, 0))
                items.append(mm_half(name, tcc, 1))
            items.append(post_item(tcc))
        return items

    # ---------------- output projection work items ----------------
    def emit_outproj_t16(b, t16):
        avT = avTs[b]
        o_sb = smalls.tile([128, 2, TC], BF16, tag="o")
        for n2 in range(D // TC):
            po = ps_scr.tile([128, TC], F32, tag="scr", name="po")
            nc.tensor.matmul(po, avT[:, t16 * 128:(t16 + 1) * 128],
                             wo_sb[:, n2 * TC:(n2 + 1) * TC],
                             start=True, stop=True)
            if (t16 + n2) % 2 == 0:
                nc.vector.tensor_copy(o_sb[:, n2, :], po)
            else:
                nc.scalar.copy(o_sb[:, n2, :], po)
        nc.sync.dma_start(
            out=out[b, t16 * 128:(t16 + 1) * 128, :],
            in_=o_sb.rearrange("p a b -> p (a b)"))

    def outproj_items(b):
        return [lambda b=b, t16=t16: emit_outproj_t16(b, t16)
                for t16 in range(seq // 128)]

    # ---------------- attention ----------------
    av_state = {}

    def emit_attn_pair(b, qc, g):
        """One strip-pair (strips 2g, 2g+1) of attention for batch b, both
        heads. j0 = higher strip (more causally trimmed) so the exp range is
        one contiguous tail slice."""
        qT, kT, vext = qTs[b], kTs[b], vexts[b]
        nstrips = 4 * qc + 4
        # j0 = strip 2g+1 (larger r), j1 = strip 2g
        order = []
        for j, st in ((0, 2 * g + 1), (1, 2 * g)):
            r = st * KS - qc * TC
            order.append((j, st, max(r, 0), r >= 0))
        # one PSUM tile per STRIP covering both heads: a single release event
        # (the strip's exp) frees both heads' banks together, so the next
        # pair's row-tiled head matmuls become ready simultaneously and join.
        pps = {}
        p_sbs = {}
        for j, st, r, diag in order:
            pps[j] = ps_p.tile([128, 2, TC], F32, tag="pp", name=f"pp{j}")
            for h in range(HPC):
                nc.tensor.matmul(
                    pps[j][:, h, r:],
                    kT[h * 64:(h + 1) * 64, st * KS:(st + 1) * KS],
                    qT[h * 64:(h + 1) * 64, qc * TC + r:(qc + 1) * TC],
                    start=True, stop=True)
            p_sb = ppool.tile([128, 2, TC], BF16, tag="p", name=f"p{j}")
            p_sbs[j] = p_sb
            nc.scalar.activation(
                p_sb.rearrange("p a b -> p (a b)"),
                pps[j].rearrange("p a b -> p (a b)"),
                mybir.ActivationFunctionType.Exp)
            if diag:  # triangular block multiply on the diagonal 128 cols
                for h in range(HPC):
                    nc.vector.tensor_mul(p_sb[:, h, r:r + KS],
                                         p_sb[:, h, r:r + KS], mask_sb)
        # For the first pair of a qc, issue the full-width strip (2g) first so
        # start=True covers every column of the accumulator.
        av_order = order[::-1] if g == 0 else order
        for j, st, r, diag in av_order:
            first = av_state[(b, qc)] == 0
            av_state[(b, qc)] += 1
            last_issue = av_state[(b, qc)] == nstrips
            for h in range(HPC):
                nc.tensor.matmul(pav_cur[h][:, r:], vext[:, h, st, :],
                                 p_sbs[j][:, h, r:],
                                 start=first, stop=last_issue)

    def emit_qc_norm(b, qc, split_mul=False):
        """Per-(head) softmax normalization for this query chunk. With
        split_mul, the final multiply is emitted in 128-col pieces so each
        downstream outproj block starts as soon as its columns are ready."""
        avT = avTs[b]
        for h in range(HPC):
            ave = smalls.tile([65, TC], F32, tag="ave")
            nc.vector.tensor_copy(ave, pav_cur[h])
            z0 = smalls.tile([1, TC], F32, tag="z0")
            nc.sync.dma_start(out=z0, in_=ave[64:65, :])
            zbb = smalls.tile([64, TC], F32, tag="zbb")
            nc.gpsimd.partition_broadcast(zbb, z0)
            rz = smalls.tile([64, TC], F32, tag="rz")
            nc.vector.reciprocal_approx_fast(rz, zbb)
            pieces = range(0, TC, 128) if split_mul else (0,)
            width = 128 if split_mul else TC
            with nc.allow_low_precision(reason="attn weights tolerate bf16"):
                for c0 in pieces:
                    nc.vector.tensor_mul(
                        avT[h * 64:(h + 1) * 64,
                            qc * TC + c0:qc * TC + c0 + width],
                        ave[0:64, c0:c0 + width], rz[:, c0:c0 + width])

    # ---------------- main schedule ----------------
    # batch 0 projections run solo up front
    alloc_batch(0)
    for it in proj_items(0):
        it()

    items = deque()
    deferred = []
    for b in range(b_count):
        avTs[b] = avpool.tile([128, seq], BF16, tag="avT", name=f"avT{b}")
        items.extend(deferred)
        deferred = []
        if b + 1 < b_count:
            alloc_batch(b + 1)
            items.extend(proj_items(b + 1))
        last = b == b_count - 1
        n_items = len(items)
        total_pairs = sum(2 * qc + 2 for qc in range(NQC))
        pair_idx = 0
        popped = 0
        for qc in range(NQC):
            av_state[(b, qc)] = 0
            global pav_cur
            pav_cur = {h: ps_av.tile([65, TC], F32, tag="av", name=f"pav{h}")
                       for h in range(HPC)}
            for g in range(2 * qc + 2):
                emit_attn_pair(b, qc, g)
                pair_idx += 1
                npairs = 2 * qc + 2
                if last and qc > 0 and g >= npairs - 4:
                    # previous qc's outproj folds into this qc's tail pairs:
                    # it keeps the PE warm through the norm-chain bubble
                    t4 = g - (npairs - 4)
                    emit_outproj_t16(b, (qc - 1) * (TC // 128) + t4)
                target = n_items * pair_idx // (total_pairs + 8)
                while popped < target and items:
                    items.popleft()()
                    popped += 1
            emit_qc_norm(b, qc, split_mul=last)
        while items:  # safety drain
            items.popleft()()
        if last:
            for t4 in range(TC // 128):
                emit_outproj_t16(b, (NQC - 1) * (TC // 128) + t4)
        if not last:
            items.extend(outproj_items(b))


def host_inputs(x, Wq, Wk, Wv, Wo, core, xt_bf=None):
    """Build the per-core input map."""
    hs = slice(core * DH, (core + 1) * DH)
    if xt_bf is None:
        xt_bf = np.ascontiguousarray(np.transpose(x, (0, 2, 1))).astype(NPBF16)
    wq = np.ascontiguousarray((Wq[hs, :] * np.float32(1.0 / np.sqrt(HD))).T).astype(NPBF16)
    wk = np.ascontiguousarray(Wk[hs, :].T).astype(NPBF16)
    wv = np.ascontiguousarray(Wv[hs, :].T).astype(NPBF16)
    wo = np.ascontiguousarray(Wo[:, hs].T).astype(NPBF16)
    # lower-triangular inclusive [128,128]: valid (q_local >= k_local)
    mask = (np.arange(128)[None, :] >= np.arange(128)[:, None]).astype(NPBF16)
    return {"xt": xt_bf, "wq": wq, "wk": wk, "wv": wv, "wo": wo, "mask": mask}


def build_program(b_count=B, seq=S):
    nc = bacc.Bacc("TRN2", target_bir_lowering=False, debug=False,
                   num_devices=NCORES)
    aps = {
        "xt": nc.dram_tensor("xt", [b_count, D, seq], BF16, kind="ExternalInput").ap(),
        "wq": nc.dram_tensor("wq", [D, DH], BF16, kind="ExternalInput").ap(),
        "wk": nc.dram_tensor("wk", [D, DH], BF16, kind="ExternalInput").ap(),
        "wv": nc.dram_tensor("wv", [D, DH], BF16, kind="ExternalInput").ap(),
        "wo": nc.dram_tensor("wo", [DH, D], BF16, kind="ExternalInput").ap(),
        "mask": nc.dram_tensor("mask", [128, 128], BF16, kind="ExternalInput").ap(),
        "out": nc.dram_tensor("out", [b_count, seq, D], BF16, kind="ExternalOutput").ap(),
    }
    with tile.TileContext(nc) as tcx:
        with ExitStack() as ctx:
            emit(tcx, ctx, aps, b_count, seq)
    nc.finalize()
    return nc


def _ensure_ntff_hook():
    """Register the ctypes NTFF profile hook when the image lacks
    antenv.axon_hooks (needed only for trace=True)."""
    import sys, types
    try:
        import antenv.axon_hooks  # noqa: F401
        return
    except ImportError:
        pass
    try:
        import antenv
        from trn_agent_boot.trn_boot import _ntff_profile_via_ctypes
        hook = _ntff_profile_via_ctypes("/opt/axon/libaxon_pjrt.so")
        mod = types.ModuleType("antenv.axon_hooks")
        mod.get_axon_ntff_profile_hook = lambda: hook
        mod.set_axon_ntff_profile_hook = lambda h: None
        sys.modules["antenv.axon_hooks"] = mod
        antenv.axon_hooks = mod
    except Exception:
        pass


def kernel(x, Wq, Wk, Wv, Wo):
    global last_exec_time_ns
    x = np.asarray(x, dtype=np.float32)
    Wq = np.asarray(Wq, dtype=np.float32)
    Wk = np.asarray(Wk, dtype=np.float32)
    Wv = np.asarray(Wv, dtype=np.float32)
    Wo = np.asarray(Wo, dtype=np.float32)

    nc = build_program(B, S)
    xt_bf = np.ascontiguousarray(np.transpose(x, (0, 2, 1))).astype(NPBF16)
    in_maps = [host_inputs(x, Wq, Wk, Wv, Wo, c, xt_bf=xt_bf) for c in range(NCORES)]
    trace = bool(os.environ.get("BASS_TRACE"))
    if trace:
        _ensure_ntff_hook()
    res = run_bass_kernel_spmd(nc, in_maps, list(range(NCORES)), trace=trace)
    last_exec_time_ns = res.exec_time_ns
    parts = [res.results[c]["out"] for c in range(NCORES)]
    acc = parts[0].astype(np.float32)
    for p in parts[1:]:
        acc = acc + p
    return acc


# revision 53
# speedup vs baseline: 1.1829x; 1.0232x over previous
"""Causal multi-head attention (B=4, S=2048, D=1024, H=16, Hd=64) on 8 TRN2
NeuronCores.

Sharding: tensor-parallel over heads. Core c owns heads [2c, 2c+1]:
  - Wq/Wk/Wv column-sharded (rows of the [out,in] weight): each core projects
    x -> qT/kT/vT [128, S] (2 heads x 64, head-dim-major).
  - Attention per (b, h) computed entirely on-core, scoresT layout
    [keys, queries] so softmax normalization folds into matmuls (ones row in
    the extended V gives the denominator for free).
  - Wo row-sharded: each core emits a partial [B,S,D] output; host sums the
    8 partials.

Perf structure:
  - Causal trimming: for diagonal key-strips only the valid query range
    [r, 512) is computed in scores / streamed into AV; the mask is a single
    [128,128] lower-triangular block multiply.
  - Scores PSUM is per-STRIP [128, 2(heads), 512]: one release event (the
    strip's exp) frees both heads' banks together, so the next pair's
    row-tiled head matmuls become ready simultaneously and run concurrently
    on disjoint PE row groups.
  - Projection and output-projection matmuls are emitted as small work items
    woven between attention strip-pairs, so the PE never idles while ACT
    works through the exp stream; a few items are held in reserve to bridge
    batch boundaries (keeps the PE HAM clock at 2.4GHz).
  - Last batch: outproj folds are delayed one query-chunk and spread over
    the tail pairs; the final normalization multiplies are split into
    128-col pieces so outproj starts per-block.

Numerics: matmul operands in bf16 (fp32 PSUM accumulation), softmax without
max-subtraction (scores are bounded ~|10| for this data distribution), causal
mask applied post-exp as a {0,1} triangular multiply.
"""

import os
from collections import deque
import numpy as np
import ml_dtypes
from contextlib import ExitStack

import concourse.bass as bass
import concourse.tile as tile
from concourse import bacc, mybir
from concourse.bass_utils import run_bass_kernel_spmd
from concourse.masks import make_identity

F32 = mybir.dt.float32
BF16 = mybir.dt.bfloat16
NPBF16 = ml_dtypes.bfloat16

B, S, D = 4, 2048, 1024
H, HD = 16, 64
NCORES = 8
HPC = H // NCORES          # heads per core
DH = HPC * HD              # local head dim (128)
TC = 512                   # token chunk for projections / query chunk
KS = 128                   # key strip

last_exec_time_ns = None   # set by kernel() when BASS_TRACE=1


def emit(tc_ctx: tile.TileContext, ctx: ExitStack, aps: dict, b_count: int, seq: int):
    """Emit the per-core program. aps: xt [b,D,seq] bf16, wq/wk/wv [D,DH] bf16,
    wo [DH,D] bf16, mask [128, 128] bf16 (lower-tri incl), out [b,seq,D] bf16."""
    nc = tc_ctx.nc
    tc = tc_ctx
    KC = D // 128            # contraction chunks for projections
    NTC = seq // TC          # token chunks
    NQC = seq // TC          # query chunks
    NKS = seq // KS          # key strips

    xt, wq, wk, wv, wo, mask, out = (
        aps["xt"], aps["wq"], aps["wk"], aps["wv"], aps["wo"], aps["mask"], aps["out"]
    )

    wpool = ctx.enter_context(tc.tile_pool(name="wpool", bufs=1))
    xpool = ctx.enter_context(tc.tile_pool(name="xpool", bufs=4))
    qkpool = ctx.enter_context(tc.tile_pool(name="qkpool", bufs=4))
    vpool = ctx.enter_context(tc.tile_pool(name="vpool", bufs=2))
    ppool = ctx.enter_context(tc.tile_pool(name="ppool", bufs=4))
    avpool = ctx.enter_context(tc.tile_pool(name="avpool", bufs=4))
    smalls = ctx.enter_context(tc.tile_pool(name="smalls", bufs=4))

    ps_scr = ctx.enter_context(tc.tile_pool(name="ps_scr", bufs=2, space="PSUM"))
    ps_p = ctx.enter_context(tc.tile_pool(name="ps_p", bufs=2, space="PSUM"))
    ps_av = ctx.enter_context(tc.tile_pool(name="ps_av", bufs=2, space="PSUM"))

    # --- first input chunk ASAP: the xt (b0, tcc0) DMAs gate the first matmul,
    # so they go out before the ~26 weight/mask dma_starts (each dma_start
    # costs ~0.6us of serial sequencing).
    xt_tiles = {}

    def emit_xt_dma(b, tcc, split=False):
        xt_src = xt[b].rearrange("(kc p) t -> p kc t", p=128)
        ts = []
        for kc in range(KC):
            t = xpool.tile([128, TC], BF16, tag="xt",
                           name=f"xt_{b}_{tcc}_{kc}", bufs=20)
            eng = nc.scalar if (split and kc % 2) else nc.sync
            eng.dma_start(out=t, in_=xt_src[:, kc, tcc * TC:(tcc + 1) * TC])
            ts.append(t)
        xt_tiles[(b, tcc)] = ts

    emit_xt_dma(0, 0, split=True)

    # --- constants / weights: per-(name, kc-half) tiles [128,4,DH]; six
    # 128KB DMAs instead of 24 small ones keeps sequencing cost low while
    # letting the first half of each name land early ---
    w_sb = {}
    for half in range(2):
        for name, ap in (("wq", wq), ("wk", wk), ("wv", wv)):
            t = wpool.tile([128, 4, DH], BF16, tag=f"w_{name}_{half}",
                           name=f"w_{name}_{half}")
            src_ap = ap.rearrange("(kc p) m -> p kc m", p=128)
            nc.scalar.dma_start(out=t, in_=src_ap[:, half * 4:half * 4 + 4, :])
            w_sb[(name, half)] = t
    mask_sb = wpool.tile([128, 128], BF16)
    nc.scalar.dma_start(out=mask_sb, in_=mask)
    wo_sb = wpool.tile([128, D], BF16)
    nc.scalar.dma_start(out=wo_sb, in_=wo)
    for _tcc in range(1, NTC):
        emit_xt_dma(0, _tcc, split=True)

    ident_f = wpool.tile([128, 64], F32)
    make_identity(nc, ident_f[0:64, :])
    make_identity(nc, ident_f[64:128, :])
    ident = wpool.tile([128, 64], BF16)
    nc.vector.tensor_copy(ident, ident_f)

    ones_f = wpool.tile([128, 64], F32)
    nc.vector.memset(ones_f, 1.0)
    ones_r = wpool.tile([128, 64], BF16)
    nc.vector.tensor_copy(ones_r, ones_f)


    qTs, kTs, vexts, avTs = {}, {}, {}, {}

    def alloc_batch(b):
        qTs[b] = qkpool.tile([128, seq], BF16, tag="qT", name=f"qT{b}")
        kTs[b] = qkpool.tile([128, seq], BF16, tag="kT", name=f"kT{b}")
        vexts[(b, "vT")] = vpool.tile([128, seq], BF16, tag="vT", name=f"vT{b}")
        vexts[b] = vpool.tile([128, HPC, NKS, 65], BF16, tag="vext",
                              name=f"vext{b}", bufs=4)
        # static ones column (softmax denominator row) written once at alloc,
        # so AV matmuls never wait on the last projection chunk
        for h in range(HPC):
            nc.vector.tensor_copy(vexts[b][:, h, :, 64:65],
                                  ones_r[:, 0:1].to_broadcast([128, NKS, 1]))

    # ---------------- projection work items ----------------
    def proj_items(b):
        """Work-item closures for batch b's projections: DMA, 4-MM bursts,
        casts, v-transposes. Each item is a small dense PE burst."""
        items = []
        ps_live = {}

        def mm_half(name, tcc, half, b=b):
            def run():
                xs = xt_tiles[(b, tcc)]
                if half == 0:
                    ps = ps_scr.tile([128, TC], F32, tag="scr", name=f"ps_{name}")
                    ps_live[(name, tcc)] = ps
                else:
                    ps = ps_live.pop((name, tcc))
                for kc in range(half * 4, half * 4 + 4):
                    nc.tensor.matmul(ps, w_sb[(name, half)][:, kc % 4, :], xs[kc],
                                     start=(kc == 0), stop=(kc == KC - 1))
                if half == 1:
                    dstmap = {"wq": qTs[b], "wk": kTs[b], "wv": vexts[(b, "vT")]}
                    # high priority: this cast releases the PSUM slot the next
                    # projection burst needs — don't let it queue behind other
                    # DVE work
                    with tc.high_priority(offset=30):
                        nc.vector.tensor_copy(
                            dstmap[name][:, tcc * TC:(tcc + 1) * TC], ps)
            return run

        def post_item(tcc, b=b):
            def run():
                vT = vexts[(b, "vT")]
                vext = vexts[b]
                for h in range(HPC):
                    tr4 = ps_scr.tile([128, 4, 64], BF16, tag="scr", name="tr4")
                    for i in range(4):
                        ks = tcc * 4 + i
                        nc.tensor.transpose(
                            tr4[:, i, :],
                            vT[h * 64:(h + 1) * 64, ks * 128:(ks + 1) * 128],
                            ident[h * 64:(h + 1) * 64, :])
                    nc.vector.tensor_copy(vext[:, h, tcc * 4:(tcc + 1) * 4, 0:64], tr4)
            return run

        for tcc in range(NTC):
            if b != 0:  # batch 0's xt DMAs all go out at program start
                items.append(lambda b=b, tcc=tcc: emit_xt_dma(b, tcc))
            for name in ("wq", "wk", "wv"):
                items.append(mm_half(name, tcc, 0))
                items.append(mm_half(name, tcc, 1))
            items.append(post_item(tcc))
        return items

    # ---------------- output projection work items ----------------
    def emit_outproj_t16(b, t16):
        avT = avTs[b]
        o_sb = smalls.tile([128, 2, TC], BF16, tag="o")
        for n2 in range(D // TC):
            po = ps_scr.tile([128, TC], F32, tag="scr", name="po")
            nc.tensor.matmul(po, avT[:, t16 * 128:(t16 + 1) * 128],
                             wo_sb[:, n2 * TC:(n2 + 1) * TC],
                             start=True, stop=True)
            if (t16 + n2) % 2 == 0:
                nc.vector.tensor_copy(o_sb[:, n2, :], po)
            else:
                nc.scalar.copy(o_sb[:, n2, :], po)
        nc.sync.dma_start(
            out=out[b, t16 * 128:(t16 + 1) * 128, :],
            in_=o_sb.rearrange("p a b -> p (a b)"))

    def outproj_items(b):
        return [lambda b=b, t16=t16: emit_outproj_t16(b, t16)
                for t16 in range(seq // 128)]

    # ---------------- attention ----------------
    av_state = {}

    def emit_attn_pair(b, qc, g):
        """One strip-pair (strips 2g, 2g+1) of attention for batch b, both
        heads. j0 = higher strip (more causally trimmed) so the exp range is
        one contiguous tail slice."""
        qT, kT, vext = qTs[b], kTs[b], vexts[b]
        nstrips = 4 * qc + 4
        # j0 = strip 2g+1 (larger r), j1 = strip 2g
        order = []
        for j, st in ((0, 2 * g + 1), (1, 2 * g)):
            r = st * KS - qc * TC
            order.append((j, st, max(r, 0), r >= 0))
        # one PSUM tile per STRIP covering both heads: a single release event
        # (the strip's exp) frees both heads' banks together, so the next
        # pair's row-tiled head matmuls become ready simultaneously and join.
        pps = {}
        p_sbs = {}
        for j, st, r, diag in order:
            pps[j] = ps_p.tile([128, 2, TC], F32, tag="pp", name=f"pp{j}")
            for h in range(HPC):
                nc.tensor.matmul(
                    pps[j][:, h, r:],
                    kT[h * 64:(h + 1) * 64, st * KS:(st + 1) * KS],
                    qT[h * 64:(h + 1) * 64, qc * TC + r:(qc + 1) * TC],
                    start=True, stop=True)
            p_sb = ppool.tile([128, 2, TC], BF16, tag="p", name=f"p{j}")
            p_sbs[j] = p_sb
            nc.scalar.activation(
                p_sb.rearrange("p a b -> p (a b)"),
                pps[j].rearrange("p a b -> p (a b)"),
                mybir.ActivationFunctionType.Exp)
            if diag:  # triangular block multiply on the diagonal 128 cols
                for h in range(HPC):
                    nc.vector.tensor_mul(p_sb[:, h, r:r + KS],
                                         p_sb[:, h, r:r + KS], mask_sb)
        # For the first pair of a qc, issue the full-width strip (2g) first so
        # start=True covers every column of the accumulator.
        av_order = order[::-1] if g == 0 else order
        for j, st, r, diag in av_order:
            first = av_state[(b, qc)] == 0
            av_state[(b, qc)] += 1
            last_issue = av_state[(b, qc)] == nstrips
            for h in range(HPC):
                nc.tensor.matmul(pav_cur[h][:, r:], vext[:, h, st, :],
                                 p_sbs[j][:, h, r:],
                                 start=first, stop=last_issue)

    def emit_qc_norm(b, qc, split_mul=False):
        """Per-(head) softmax normalization for this query chunk. With
        split_mul, the final multiply is emitted in 128-col pieces so each
        downstream outproj block starts as soon as its columns are ready."""
        avT = avTs[b]
        for h in range(HPC):
            ave = smalls.tile([65, TC], F32, tag="ave")
            nc.vector.tensor_copy(ave, pav_cur[h])
            z0 = smalls.tile([1, TC], F32, tag="z0")
            nc.sync.dma_start(out=z0, in_=ave[64:65, :])
            zbb = smalls.tile([64, TC], F32, tag="zbb")
            nc.gpsimd.partition_broadcast(zbb, z0)
            rz = smalls.tile([64, TC], F32, tag="rz")
            nc.vector.reciprocal_approx_fast(rz, zbb)
            pieces = range(0, TC, 128) if split_mul else (0,)
            width = 128 if split_mul else TC
            with nc.allow_low_precision(reason="attn weights tolerate bf16"):
                for c0 in pieces:
                    nc.vector.tensor_mul(
                        avT[h * 64:(h + 1) * 64,
                            qc * TC + c0:qc * TC + c0 + width],
                        ave[0:64, c0:c0 + width], rz[:, c0:c0 + width])

    # ---------------- main schedule ----------------
    # batch 0 projections run solo up front
    alloc_batch(0)
    for it in proj_items(0):
        it()

    items = deque()
    deferred = []
    for b in range(b_count):
        avTs[b] = avpool.tile([128, seq], BF16, tag="avT", name=f"avT{b}")
        items.extend(deferred)
        deferred = []
        if b + 1 < b_count:
            alloc_batch(b + 1)
            items.extend(proj_items(b + 1))
        last = b == b_count - 1
        n_items = len(items)
        total_pairs = sum(2 * qc + 2 for qc in range(NQC))
        pair_idx = 0
        popped = 0
        for qc in range(NQC):
            av_state[(b, qc)] = 0
            global pav_cur
            pav_cur = {h: ps_av.tile([65, TC], F32, tag="av", name=f"pav{h}")
                       for h in range(HPC)}
            for g in range(2 * qc + 2):
                emit_attn_pair(b, qc, g)
                pair_idx += 1
                npairs = 2 * qc + 2
                if last and qc > 0 and g >= npairs - 4:
                    # previous qc's outproj folds into this qc's tail pairs:
                    # it keeps the PE warm through the norm-chain bubble
                    t4 = g - (npairs - 4)
                    emit_outproj_t16(b, (qc - 1) * (TC // 128) + t4)
                target = n_items * pair_idx // (total_pairs + 5)
                while popped < target and items:
                    items.popleft()()
                    popped += 1
            emit_qc_norm(b, qc, split_mul=last)
        while items:  # safety drain
            items.popleft()()
        if last:
            for t4 in range(TC // 128):
                emit_outproj_t16(b, (NQC - 1) * (TC // 128) + t4)
        if not last:
            items.extend(outproj_items(b))


def host_inputs(x, Wq, Wk, Wv, Wo, core, xt_bf=None):
    """Build the per-core input map."""
    hs = slice(core * DH, (core + 1) * DH)
    if xt_bf is None:
        xt_bf = np.ascontiguousarray(np.transpose(x, (0, 2, 1))).astype(NPBF16)
    wq = np.ascontiguousarray((Wq[hs, :] * np.float32(1.0 / np.sqrt(HD))).T).astype(NPBF16)
    wk = np.ascontiguousarray(Wk[hs, :].T).astype(NPBF16)
    wv = np.ascontiguousarray(Wv[hs, :].T).astype(NPBF16)
    wo = np.ascontiguousarray(Wo[:, hs].T).astype(NPBF16)
    # lower-triangular inclusive [128,128]: valid (q_local >= k_local)
    mask = (np.arange(128)[None, :] >= np.arange(128)[:, None]).astype(NPBF16)
    return {"xt": xt_bf, "wq": wq, "wk": wk, "wv": wv, "wo": wo, "mask": mask}


def build_program(b_count=B, seq=S):
    nc = bacc.Bacc("TRN2", target_bir_lowering=False, debug=False,
                   num_devices=NCORES)
    aps = {
        "xt": nc.dram_tensor("xt", [b_count, D, seq], BF16, kind="ExternalInput").ap(),
        "wq": nc.dram_tensor("wq", [D, DH], BF16, kind="ExternalInput").ap(),
        "wk": nc.dram_tensor("wk", [D, DH], BF16, kind="ExternalInput").ap(),
        "wv": nc.dram_tensor("wv", [D, DH], BF16, kind="ExternalInput").ap(),
        "wo": nc.dram_tensor("wo", [DH, D], BF16, kind="ExternalInput").ap(),
        "mask": nc.dram_tensor("mask", [128, 128], BF16, kind="ExternalInput").ap(),
        "out": nc.dram_tensor("out", [b_count, seq, D], BF16, kind="ExternalOutput").ap(),
    }
    with tile.TileContext(nc) as tcx:
        with ExitStack() as ctx:
            emit(tcx, ctx, aps, b_count, seq)
    nc.finalize()
    return nc


def _ensure_ntff_hook():
    """Register the ctypes NTFF profile hook when the image lacks
    antenv.axon_hooks (needed only for trace=True)."""
    import sys, types
    try:
        import antenv.axon_hooks  # noqa: F401
        return
    except ImportError:
        pass
    try:
        import antenv
        from trn_agent_boot.trn_boot import _ntff_profile_via_ctypes
        hook = _ntff_profile_via_ctypes("/opt/axon/libaxon_pjrt.so")
        mod = types.ModuleType("antenv.axon_hooks")
        mod.get_axon_ntff_profile_hook = lambda: hook
        mod.set_axon_ntff_profile_hook = lambda h: None
        sys.modules["antenv.axon_hooks"] = mod
        antenv.axon_hooks = mod
    except Exception:
        pass


def kernel(x, Wq, Wk, Wv, Wo):
    global last_exec_time_ns
    x = np.asarray(x, dtype=np.float32)
    Wq = np.asarray(Wq, dtype=np.float32)
    Wk = np.asarray(Wk, dtype=np.float32)
    Wv = np.asarray(Wv, dtype=np.float32)
    Wo = np.asarray(Wo, dtype=np.float32)

    nc = build_program(B, S)
    xt_bf = np.ascontiguousarray(np.transpose(x, (0, 2, 1))).astype(NPBF16)
    in_maps = [host_inputs(x, Wq, Wk, Wv, Wo, c, xt_bf=xt_bf) for c in range(NCORES)]
    trace = bool(os.environ.get("BASS_TRACE"))
    if trace:
        _ensure_ntff_hook()
    res = run_bass_kernel_spmd(nc, in_maps, list(range(NCORES)), trace=trace)
    last_exec_time_ns = res.exec_time_ns
    parts = [res.results[c]["out"] for c in range(NCORES)]
    acc = parts[0].astype(np.float32)
    for p in parts[1:]:
        acc = acc + p
    return acc
